# revision 14
# baseline (speedup 1.0000x reference)
"""AnnularDilatedKNN on 8 TRN2 NeuronCores.

Problem: for each of B=4 batches of N=4096 points (xyz scaled ~N(0,30^2)),
ball-query (radius 16, nsample 32) keeps the first-32 in-ball indices per
query point (ascending index order, padded with the first hit); the dilated
selection keeps ranks {0} u [16,30] (0-based) -> 16 ids per point; gather
xyz (3ch) and feature (64ch) at those ids, channel-major outputs.

Sharding: 8 cores = 4 batches x 2 row-halves. Each core owns a
[2048 rows x 4096 cols] slab of the distance matrix.

Per-core pipeline (per 128-row tile):
  1. PE matmul (bf16 K=30 exact 3-way split of fp32):
     dotadj[r,c] = x_r.x_c - 0.5|x_c|^2 with bf16 h1/h2/h3 splits; every
     product is exact in fp32 and PSUM accumulates fp32, reproducing the
     CPU-XLA fp32 in-ball decisions bit-for-bit on this dataset (verified).
     in-ball  <=>  dotadj > 0.5(|x_r|^2 - R^2)  (thr_r, per-partition scalar).
  2. ScalarE Sign activation with bias=-thr_r -> s in {-1,+1} (bf16).
  3. DVE tensor_tensor_scan: state = (s + state) + 1  ->  g2 = 2*cumsum(inball).
  4. DVE max_index on g2 searching constants {2,34,36,..,46} and {48,..,62}
     = first column where cumsum reaches rank {1, 17..23} / {24..31}.
  5. Fixups: missing rank (0xFFFF) -> center id (pad-with-first semantics).
  6. PE transpose + replicate-matmul -> idxs int16 in dma_gather's wrapped
     [128, num_idxs/16] layout.
  7. SBUF-source transpose dma_gather over a u16-plane-packed table
     (stripe layout [128 part, 32 ranks x 512B]; row idx at partition
     idx%128, stripe idx//128; first 128 u16 = low halves of the 128
     channels, next 128 = high halves). The xbar transpose then lands
     channel c's (low,high) u16 pair on partition c -> channel-major
     without any PE transposes.
  8. ScalarE re-interleave (u16 planes -> adjacent lo/hi pairs) -> bitcast
     f32 [128ch, 2048] -> SWDGE DMA (16-engine spray) to DRAM.
"""

import os
import sys

import numpy as np

if "/opt/trn_rl_repo" not in sys.path:
    sys.path.insert(0, "/opt/trn_rl_repo")

import ml_dtypes  # noqa: E402

import concourse.bass as bass  # noqa: E402
import concourse.bacc as bacc  # noqa: E402
import concourse.mybir as mybir  # noqa: E402
import concourse.tile as tile  # noqa: E402
from concourse import library_config  # noqa: E402
from concourse.bass_utils import run_bass_kernel_spmd  # noqa: E402

F32 = mybir.dt.float32
BF16 = mybir.dt.bfloat16
I16 = mybir.dt.int16
U16 = mybir.dt.uint16

B = 4
N = 4096
CFEAT = 64
K = 16  # output ids per point
ELEM = 128  # packed table row channels (64 feat + 3 xyz + 61 pad)
N_CORES = 8
ROWS = N // 2  # rows per core
PT = 128  # partition tile
KSPLIT = 30  # 3 coords x 3x3 bf16 split products + 3 rows for -0.5|x_c|^2
RADIUS2 = 256.0

# searched scan values: 2*rank for ranks {1, 17..23} then {24..31}
SEARCH_VALS = [2.0] + [2.0 * r for r in range(17, 24)] + [2.0 * r for r in range(24, 32)]
assert len(SEARCH_VALS) == 16

LAST_RESULTS = None  # BassKernelResults of the most recent run (for test.py)


def build_nc(rows=ROWS, n=N):
    """Build the per-core Bass graph. All 8 cores run this same program."""
    nt = rows // PT  # row tiles
    ncc = n // 512  # 512-col matmul chunks
    nrank = n // PT  # table stripes
    assert rows % PT == 0 and n % 512 == 0

    nc = bacc.Bacc("TRN2", target_bir_lowering=False, debug=False)

    # ---- per-core DRAM parameters ----
    w30 = nc.dram_tensor("w30", [KSPLIT, rows], BF16, kind="ExternalInput")
    b30 = nc.dram_tensor("b30", [KSPLIT, n], BF16, kind="ExternalInput")
    negthr = nc.dram_tensor("negthr", [PT, nt], F32, kind="ExternalInput")
    table = nc.dram_tensor("table", [PT, nrank * 2 * ELEM], U16, kind="ExternalInput")
    vals_in = nc.dram_tensor("vals", [PT, 16], F32, kind="ExternalInput")
    rep_in = nc.dram_tensor("rep", [16, PT], F32, kind="ExternalInput")
    ident_in = nc.dram_tensor("ident", [PT, PT], F32, kind="ExternalInput")
    out_d = nc.dram_tensor("out", [67, rows * K], F32, kind="ExternalOutput")

    with tile.TileContext(nc) as tc:
        with (
            tc.tile_pool(name="const", bufs=1) as cpool,
            tc.tile_pool(name="s", bufs=2) as spool,
            tc.tile_pool(name="g", bufs=2) as gpool,
            tc.tile_pool(name="small", bufs=4) as fpool,
            tc.tile_pool(name="idx", bufs=2) as ipool,
            tc.tile_pool(name="gath", bufs=3) as gapool,
            tc.tile_pool(name="outc", bufs=3) as opool,
            tc.tile_pool(name="mm", bufs=6, space="PSUM") as mmpool,
            tc.tile_pool(name="tp1", bufs=1, space="PSUM") as tp1pool,
            tc.tile_pool(name="tp2", bufs=1, space="PSUM") as tp2pool,
        ):
            nc.gpsimd.load_library(library_config.mlp)

            # ---- load constants ----
            w_sb = cpool.tile([KSPLIT, rows], BF16)
            nc.sync.dma_start(out=w_sb[:], in_=w30[:])
            b_sb = cpool.tile([KSPLIT, n], BF16)
            nc.sync.dma_start(out=b_sb[:], in_=b30[:])
            thr_sb = cpool.tile([PT, nt], F32)
            nc.sync.dma_start(out=thr_sb[:], in_=negthr[:])
            valsf_sb = cpool.tile([PT, 16], F32)
            nc.sync.dma_start(out=valsf_sb[:], in_=vals_in[:])
            rep_sb = cpool.tile([16, PT], F32)
            nc.sync.dma_start(out=rep_sb[:], in_=rep_in[:])
            ident_sb = cpool.tile([PT, PT], F32)
            nc.sync.dma_start(out=ident_sb[:], in_=ident_in[:])
            tab_sb = cpool.tile([PT, nrank * 2 * ELEM], U16)
            nc.sync.dma_start(out=tab_sb[:], in_=table[:])

            vals_bf = cpool.tile([PT, 16], BF16)
            nc.vector.tensor_copy(vals_bf[:], valsf_sb[:])
            ones_sb = cpool.tile([PT, n], BF16)
            nc.gpsimd.memset(ones_sb[:], 1.0)
            zeros16 = cpool.tile([PT, 16], F32)
            nc.gpsimd.memset(zeros16[:], 0.0)

            pend_g = []  # (ix tile, t) awaiting gather
            pend_s = []  # (gtu tile, t) awaiting interleave+store

            def tail_gather(ix_t, t):
                # ---- 7: SBUF-source transpose gather (u16 planes) ----
                gtu = gapool.tile([PT, 2, PT * K], U16, tag="gtu")
                nc.gpsimd.dma_gather(
                    gtu[:],
                    tab_sb[:],
                    ix_t[:],
                    PT * K,
                    PT * K,
                    2 * ELEM,
                    transpose=True,
                    single_packet=False,
                    sbuf_tokens_per_rank=PT,
                    sbuf_free_dim_per_rank=2 * ELEM * 2,
                    sbuf_free_dim_pad_per_rank=0,
                    sbuf_byte_offset=0,
                )
                pend_s.append((gtu, t))

            def tail_store(gtu, t):
                # ---- 8: re-interleave u16 planes -> f32, DMA out ----
                outu = opool.tile([PT, 2 * PT * K], U16, tag="outu")
                nc.scalar.activation(
                    outu[:],
                    gtu[:].rearrange("p f i -> p i f"),
                    mybir.ActivationFunctionType.Copy,
                )
                nc.gpsimd.dma_start(
                    out=out_d[:, t * PT * K : (t + 1) * PT * K],
                    in_=outu[0:67, :].bitcast(F32),
                )

            for t in range(nt):
                # ---- 1+2: dot-product matmul chunks + Sign threshold ----
                s_sb = spool.tile([PT, n], BF16, tag="s")
                wT = w_sb[:, t * PT : (t + 1) * PT]
                for c in range(ncc):
                    mm = mmpool.tile([PT, 512], F32, tag="mm")
                    nc.tensor.matmul(
                        mm[:],
                        lhsT=wT,
                        rhs=b_sb[:, c * 512 : (c + 1) * 512],
                        start=True,
                        stop=True,
                    )
                    nc.scalar.activation(
                        s_sb[:, c * 512 : (c + 1) * 512],
                        mm[:],
                        mybir.ActivationFunctionType.Sign,
                        bias=thr_sb[:, t : t + 1],
                        scale=1.0,
                    )

                # ---- 3: g2 = 2*cumsum(inball): state = (s + state) + 1 ----
                g2 = gpool.tile([PT, n], BF16, tag="g")
                nc.vector.tensor_tensor_scan(
                    g2[:],
                    s_sb[:],
                    ones_sb[:],
                    0.0,
                    mybir.AluOpType.add,
                    mybir.AluOpType.add,
                )

                # ---- 4: rank extraction ----
                iu = fpool.tile([PT, 16], U16, tag="iu")
                nc.vector.max_index(iu[:, 0:8], vals_bf[:, 0:8], g2[:])
                nc.vector.max_index(iu[:, 8:16], vals_bf[:, 8:16], g2[:])

                # ---- 5: missing-rank padding with center, fused to 4
                # DVE ops (small DVE ops inherit multi-us drain penalties) ----
                thr_inv = float(n) - 0.5
                fx = fpool.tile([PT, 16], F32, tag="fx")
                nc.vector.tensor_copy(fx[:], iu[:])
                mc = fpool.tile([PT, 16], F32, tag="mc")
                nc.vector.tensor_scalar(
                    mc[:], fx[:], thr_inv, fx[:, 0:1],
                    mybir.AluOpType.is_gt, mybir.AluOpType.mult,
                )  # {0 | center}
                vfx = fpool.tile([PT, 16], F32, tag="vfx")
                nc.vector.scalar_tensor_tensor(
                    vfx[:], fx[:], thr_inv, fx[:],
                    mybir.AluOpType.is_le, mybir.AluOpType.mult,
                )  # {id | 0}
                fx2 = fpool.tile([PT, 16], F32, tag="fx2")
                nc.vector.tensor_tensor(fx2[:], vfx[:], mc[:], mybir.AluOpType.add)

                # ---- 6: idxs into wrapped [128, 128] int16 layout ----
                tp1 = tp1pool.tile([16, PT], F32, tag="tp1")
                nc.tensor.transpose(tp1[:], fx2[:], ident_sb[:])
                t1 = fpool.tile([16, PT], F32, tag="t1")
                nc.scalar.activation(
                    t1[:], tp1[:], mybir.ActivationFunctionType.Copy
                )
                tp2 = tp2pool.tile([PT, PT], F32, tag="tp2")
                nc.tensor.matmul(tp2[:], lhsT=rep_sb[:], rhs=t1[:], start=True, stop=True)
                ix = ipool.tile([PT, PT], I16, tag="ix")
                nc.scalar.activation(
                    ix[:], tp2[:], mybir.ActivationFunctionType.Copy
                )

                pend_g.append((ix, t))
                if len(pend_g) > 1:
                    tail_gather(*pend_g.pop(0))
                if len(pend_s) > 1:
                    tail_store(*pend_s.pop(0))

            while pend_g:
                tail_gather(*pend_g.pop(0))
            while pend_s:
                tail_store(*pend_s.pop(0))

    nc.compile()
    return nc


def _split3(x):
    """Exact 3-way bf16 split of fp32 values (h1+h2+h3 == x exactly)."""
    h1 = x.astype(ml_dtypes.bfloat16).astype(np.float32)
    h2 = (x - h1).astype(ml_dtypes.bfloat16).astype(np.float32)
    h3 = (x - h1 - h2).astype(ml_dtypes.bfloat16).astype(np.float32)
    return h1, h2, h3


def _host_prep(xyz_b, feat_b, half, rows=ROWS, n=N):
    """Per-core input map. xyz_b [n,3] f32, feat_b [n,CFEAT] f32."""
    nt = rows // PT
    nrank = n // PT
    x = np.ascontiguousarray(xyz_b.astype(np.float32))
    # squared norms with XLA-matching rounding order: (x0^2 + x1^2) + x2^2
    sq = ((x[:, 0] * x[:, 0] + x[:, 1] * x[:, 1]) + x[:, 2] * x[:, 2]).astype(
        np.float32
    )
    r0 = half * rows
    rsl = slice(r0, r0 + rows)

    # K=30 bf16 operands; PSUM accumulates k sequentially so the order here
    # defines the fp32 rounding sequence (matches the verified emulation):
    # k = 9d+3i+j -> h_i(row coord d) * h_j(col coord d); k=27..29 -> 1 * m_j
    w30 = np.zeros((KSPLIT, rows), np.float32)
    b30 = np.zeros((KSPLIT, n), np.float32)
    for d in range(3):
        hr = _split3(x[rsl, d])
        hc = _split3(x[:, d])
        for i in range(3):
            for j in range(3):
                k = 9 * d + 3 * i + j
                w30[k] = hr[i]
                b30[k] = hc[j]
    m05 = (-0.5 * sq).astype(np.float32)
    ms = _split3(m05)
    for j in range(3):
        w30[27 + j] = 1.0
        b30[27 + j] = ms[j]
    w30 = w30.astype(ml_dtypes.bfloat16)
    b30 = b30.astype(ml_dtypes.bfloat16)

    thr = (0.5 * (sq[rsl] - np.float32(RADIUS2))).astype(np.float32)
    negthr = np.ascontiguousarray((-thr).reshape(nt, PT).T)

    # u16-plane packed gather table, SBUF stripe layout:
    # row idx -> partition idx%128, stripe idx//128; stripe = 256 u16 =
    # [low halves of 128 channels | high halves of 128 channels]
    rowdata = np.zeros((n, ELEM), np.float32)
    rowdata[:, :CFEAT] = feat_b
    rowdata[:, CFEAT : CFEAT + 3] = x
    u = rowdata.view(np.uint16).reshape(n, ELEM, 2)  # little-endian lo/hi
    planes = np.concatenate([u[:, :, 0], u[:, :, 1]], axis=1)  # [n, 256]
    table = np.zeros((PT, nrank * 2 * ELEM), np.uint16)
    for r in range(nrank):
        table[:, r * 2 * ELEM : (r + 1) * 2 * ELEM] = planes[
            r * PT : (r + 1) * PT
        ]

    vals = np.broadcast_to(
        np.asarray(SEARCH_VALS, np.float32)[None, :], (PT, 16)
    ).copy()
    rep = np.zeros((16, PT), np.float32)
    rep[np.arange(PT) % 16, np.arange(PT)] = 1.0
    ident = np.eye(PT, dtype=np.float32)

    return {
        "w30": w30,
        "b30": b30,
        "negthr": negthr,
        "table": table,
        "vals": vals,
        "rep": rep,
        "ident": ident,
    }


_NC_CACHE = {}


def _get_nc():
    if "nc" not in _NC_CACHE:
        _NC_CACHE["nc"] = build_nc()
    return _NC_CACHE["nc"]


def kernel(xyz, feature):
    """Full inputs -> full outputs (dilated_xyz [B,3,N,K], dilated_feature
    [B,CFEAT,N,K]), computed on 8 NeuronCores."""
    global LAST_RESULTS
    xyz = np.asarray(xyz)
    feature = np.asarray(feature)
    assert xyz.shape == (B, N, 3) and feature.shape == (B, N, CFEAT)

    nc = _get_nc()
    in_maps = [
        _host_prep(xyz[core // 2], feature[core // 2], core % 2)
        for core in range(N_CORES)
    ]
    trace = bool(int(os.environ.get("KNN_TRACE", "0")))
    if trace:
        # the axon trace path uploads artifacts to a fish bucket; neuter
        # that (no credentials in this container) so profiling is local-only
        import concourse.bass_utils as _bu

        _bu.upload_artifacts = lambda tmpdir: str(tmpdir)
    try:
        res = run_bass_kernel_spmd(
            nc, in_maps, core_ids=list(range(N_CORES)), trace=trace
        )
    except Exception:
        if not trace:
            raise
        res = run_bass_kernel_spmd(
            nc, in_maps, core_ids=list(range(N_CORES)), trace=False
        )
    LAST_RESULTS = res

    dxyz = np.empty((B, 3, N, K), np.float32)
    dfeat = np.empty((B, CFEAT, N, K), np.float32)
    for core in range(N_CORES):
        b, half = core // 2, core % 2
        o = res.results[core]["out"]  # [67, ROWS*K]
        rs = slice(half * ROWS, (half + 1) * ROWS)
        dfeat[b, :, rs, :] = o[:CFEAT].reshape(CFEAT, ROWS, K)
        dxyz[b, :, rs, :] = o[CFEAT : CFEAT + 3].reshape(3, ROWS, K)
    return dxyz, dfeat


# revision 15
# speedup vs baseline: 1.1680x; 1.1680x over previous
"""AnnularDilatedKNN on 8 TRN2 NeuronCores.

Problem: for each of B=4 batches of N=4096 points (xyz scaled ~N(0,30^2)),
ball-query (radius 16, nsample 32) keeps the first-32 in-ball indices per
query point (ascending index order, padded with the first hit); the dilated
selection keeps ranks {0} u [16,30] (0-based) -> 16 ids per point; gather
xyz (3ch) and feature (64ch) at those ids, channel-major outputs.

Sharding: 8 cores = 4 batches x 2 row-halves. Each core owns a
[2048 rows x 4096 cols] slab of the distance matrix.

Per-core pipeline (per 128-row tile):
  1. PE matmul (bf16 K=30 exact 3-way split of fp32):
     dotadj[r,c] = x_r.x_c - 0.5|x_c|^2 with bf16 h1/h2/h3 splits; every
     product is exact in fp32 and PSUM accumulates fp32, reproducing the
     CPU-XLA fp32 in-ball decisions bit-for-bit on this dataset (verified).
     in-ball  <=>  dotadj > 0.5(|x_r|^2 - R^2)  (thr_r, per-partition scalar).
  2. ScalarE Sign activation with bias=-thr_r -> s in {-1,+1} (bf16).
  3. DVE tensor_tensor_scan: state = (s + state) + 1  ->  g2 = 2*cumsum(inball).
  4. DVE max_index on g2 searching constants {2,34,36,..,46} and {48,..,62}
     = first column where cumsum reaches rank {1, 17..23} / {24..31}.
  5. Fixups: missing rank (0xFFFF) -> center id (pad-with-first semantics).
  6. PE transpose + replicate-matmul -> idxs int16 in dma_gather's wrapped
     [128, num_idxs/16] layout.
  7. SBUF-source transpose dma_gather over a u16-plane-packed table
     (stripe layout [128 part, 32 ranks x 512B]; row idx at partition
     idx%128, stripe idx//128; first 128 u16 = low halves of the 128
     channels, next 128 = high halves). The xbar transpose then lands
     channel c's (low,high) u16 pair on partition c -> channel-major
     without any PE transposes.
  8. ScalarE re-interleave (u16 planes -> adjacent lo/hi pairs) -> bitcast
     f32 [128ch, 2048] -> SWDGE DMA (16-engine spray) to DRAM.
"""

import os
import sys

import numpy as np

if "/opt/trn_rl_repo" not in sys.path:
    sys.path.insert(0, "/opt/trn_rl_repo")

import ml_dtypes  # noqa: E402

import concourse.bass as bass  # noqa: E402
import concourse.bacc as bacc  # noqa: E402
import concourse.mybir as mybir  # noqa: E402
import concourse.tile as tile  # noqa: E402
from concourse import library_config  # noqa: E402
from concourse.bass_utils import run_bass_kernel_spmd  # noqa: E402

F32 = mybir.dt.float32
BF16 = mybir.dt.bfloat16
I16 = mybir.dt.int16
U16 = mybir.dt.uint16

B = 4
N = 4096
CFEAT = 64
K = 16  # output ids per point
ELEM = 128  # packed table row channels (64 feat + 3 xyz + 61 pad)
N_CORES = 8
ROWS = N // 2  # rows per core
PT = 128  # partition tile
KSPLIT = 30  # 3 coords x 3x3 bf16 split products + 3 rows for -0.5|x_c|^2
RADIUS2 = 256.0

# searched scan values: 2*rank for ranks {1, 17..23} then {24..31}
SEARCH_VALS = [2.0] + [2.0 * r for r in range(17, 24)] + [2.0 * r for r in range(24, 32)]
assert len(SEARCH_VALS) == 16

LAST_RESULTS = None  # BassKernelResults of the most recent run (for test.py)


def build_nc(rows=ROWS, n=N):
    """Build the per-core Bass graph. All 8 cores run this same program."""
    nt = rows // PT  # row tiles
    ncc = n // 512  # 512-col matmul chunks
    nrank = n // PT  # table stripes
    assert rows % PT == 0 and n % 512 == 0

    nc = bacc.Bacc("TRN2", target_bir_lowering=False, debug=False)

    # ---- per-core DRAM parameters ----
    w30 = nc.dram_tensor("w30", [KSPLIT, rows], BF16, kind="ExternalInput")
    b30 = nc.dram_tensor("b30", [KSPLIT, n], BF16, kind="ExternalInput")
    negthr = nc.dram_tensor("negthr", [PT, nt], F32, kind="ExternalInput")
    table = nc.dram_tensor("table", [PT, nrank * 2 * ELEM], U16, kind="ExternalInput")
    vals_in = nc.dram_tensor("vals", [PT, 16], F32, kind="ExternalInput")
    rep_in = nc.dram_tensor("rep", [16, PT], F32, kind="ExternalInput")
    ident_in = nc.dram_tensor("ident", [PT, PT], F32, kind="ExternalInput")
    out_d = nc.dram_tensor("out", [67, rows * K], F32, kind="ExternalOutput")

    with tile.TileContext(nc) as tc:
        with (
            tc.tile_pool(name="const", bufs=1) as cpool,
            tc.tile_pool(name="s", bufs=2) as spool,
            tc.tile_pool(name="g", bufs=2) as gpool,
            tc.tile_pool(name="small", bufs=4) as fpool,
            tc.tile_pool(name="idx", bufs=2) as ipool,
            tc.tile_pool(name="gath", bufs=3) as gapool,
            tc.tile_pool(name="outc", bufs=3) as opool,
            tc.tile_pool(name="mm", bufs=6, space="PSUM") as mmpool,
            tc.tile_pool(name="tp1", bufs=1, space="PSUM") as tp1pool,
            tc.tile_pool(name="tp2", bufs=1, space="PSUM") as tp2pool,
        ):
            nc.gpsimd.load_library(library_config.mlp)

            # ---- load constants ----
            w_sb = cpool.tile([KSPLIT, rows], BF16)
            nc.sync.dma_start(out=w_sb[:], in_=w30[:])
            b_sb = cpool.tile([KSPLIT, n], BF16)
            nc.sync.dma_start(out=b_sb[:], in_=b30[:])
            thr_sb = cpool.tile([PT, nt], F32)
            nc.sync.dma_start(out=thr_sb[:], in_=negthr[:])
            valsf_sb = cpool.tile([PT, 16], F32)
            nc.sync.dma_start(out=valsf_sb[:], in_=vals_in[:])
            rep_sb = cpool.tile([16, PT], F32)
            nc.sync.dma_start(out=rep_sb[:], in_=rep_in[:])
            ident_sb = cpool.tile([PT, PT], F32)
            nc.sync.dma_start(out=ident_sb[:], in_=ident_in[:])
            tab_sb = cpool.tile([PT, nrank * 2 * ELEM], U16)
            nc.sync.dma_start(out=tab_sb[:], in_=table[:])

            vals_bf = cpool.tile([PT, 16], BF16)
            nc.vector.tensor_copy(vals_bf[:], valsf_sb[:])
            ones_sb = cpool.tile([PT, n], BF16)
            nc.gpsimd.memset(ones_sb[:], 1.0)
            zeros16 = cpool.tile([PT, 16], F32)
            nc.gpsimd.memset(zeros16[:], 0.0)

            pending = []  # (ix tile, t) awaiting gather+store

            def tail(ix_t, t):
                # ---- 7: SBUF-source transpose gather (u16 planes) ----
                gtu = gapool.tile([PT, 2, PT * K], U16, tag="gtu")
                nc.gpsimd.dma_gather(
                    gtu[:],
                    tab_sb[:],
                    ix_t[:],
                    PT * K,
                    PT * K,
                    2 * ELEM,
                    transpose=True,
                    single_packet=False,
                    sbuf_tokens_per_rank=PT,
                    sbuf_free_dim_per_rank=2 * ELEM * 2,
                    sbuf_free_dim_pad_per_rank=0,
                    sbuf_byte_offset=0,
                )

                # ---- 8: re-interleave u16 planes -> f32, DMA out ----
                outu = opool.tile([PT, 2 * PT * K], U16, tag="outu")
                nc.scalar.activation(
                    outu[:],
                    gtu[:].rearrange("p f i -> p i f"),
                    mybir.ActivationFunctionType.Copy,
                )
                nc.gpsimd.dma_start(
                    out=out_d[:, t * PT * K : (t + 1) * PT * K],
                    in_=outu[0:67, :].bitcast(F32),
                )

            for t in range(nt):
                # ---- 1+2: dot-product matmul chunks + Sign threshold ----
                s_sb = spool.tile([PT, n], BF16, tag="s")
                wT = w_sb[:, t * PT : (t + 1) * PT]
                for c in range(ncc):
                    mm = mmpool.tile([PT, 512], F32, tag="mm")
                    nc.tensor.matmul(
                        mm[:],
                        lhsT=wT,
                        rhs=b_sb[:, c * 512 : (c + 1) * 512],
                        start=True,
                        stop=True,
                    )
                    nc.scalar.activation(
                        s_sb[:, c * 512 : (c + 1) * 512],
                        mm[:],
                        mybir.ActivationFunctionType.Sign,
                        bias=thr_sb[:, t : t + 1],
                        scale=1.0,
                    )

                # ---- 3: g2 = 2*cumsum(inball): state = (s + state) + 1 ----
                g2 = gpool.tile([PT, n], BF16, tag="g")
                nc.vector.tensor_tensor_scan(
                    g2[:],
                    s_sb[:],
                    ones_sb[:],
                    0.0,
                    mybir.AluOpType.add,
                    mybir.AluOpType.add,
                )

                # ---- 4: rank extraction ----
                iu = fpool.tile([PT, 16], U16, tag="iu")
                nc.vector.max_index(iu[:, 0:8], vals_bf[:, 0:8], g2[:])
                nc.vector.max_index(iu[:, 8:16], vals_bf[:, 8:16], g2[:])

                # ---- 5: missing-rank padding with center, fused to 4
                # DVE ops (small DVE ops inherit multi-us drain penalties) ----
                thr_inv = float(n) - 0.5
                fx = fpool.tile([PT, 16], F32, tag="fx")
                nc.vector.tensor_copy(fx[:], iu[:])
                mc = fpool.tile([PT, 16], F32, tag="mc")
                nc.vector.tensor_scalar(
                    mc[:], fx[:], thr_inv, fx[:, 0:1],
                    mybir.AluOpType.is_gt, mybir.AluOpType.mult,
                )  # {0 | center}
                vfx = fpool.tile([PT, 16], F32, tag="vfx")
                nc.vector.scalar_tensor_tensor(
                    vfx[:], fx[:], thr_inv, fx[:],
                    mybir.AluOpType.is_le, mybir.AluOpType.mult,
                )  # {id | 0}
                fx2 = fpool.tile([PT, 16], F32, tag="fx2")
                nc.vector.tensor_tensor(fx2[:], vfx[:], mc[:], mybir.AluOpType.add)

                # ---- 6: idxs into wrapped [128, 128] int16 layout ----
                tp1 = tp1pool.tile([16, PT], F32, tag="tp1")
                nc.tensor.transpose(tp1[:], fx2[:], ident_sb[:])
                t1 = fpool.tile([16, PT], F32, tag="t1")
                nc.scalar.activation(
                    t1[:], tp1[:], mybir.ActivationFunctionType.Copy
                )
                tp2 = tp2pool.tile([PT, PT], F32, tag="tp2")
                nc.tensor.matmul(tp2[:], lhsT=rep_sb[:], rhs=t1[:], start=True, stop=True)
                ix = ipool.tile([PT, PT], I16, tag="ix")
                nc.scalar.activation(
                    ix[:], tp2[:], mybir.ActivationFunctionType.Copy
                )

                pending.append((ix, t))
                if len(pending) > 1:
                    tail(*pending.pop(0))

            while pending:
                tail(*pending.pop(0))

    nc.compile()
    return nc


def _split3(x):
    """Exact 3-way bf16 split of fp32 values (h1+h2+h3 == x exactly)."""
    h1 = x.astype(ml_dtypes.bfloat16).astype(np.float32)
    h2 = (x - h1).astype(ml_dtypes.bfloat16).astype(np.float32)
    h3 = (x - h1 - h2).astype(ml_dtypes.bfloat16).astype(np.float32)
    return h1, h2, h3


def _host_prep(xyz_b, feat_b, half, rows=ROWS, n=N):
    """Per-core input map. xyz_b [n,3] f32, feat_b [n,CFEAT] f32."""
    nt = rows // PT
    nrank = n // PT
    x = np.ascontiguousarray(xyz_b.astype(np.float32))
    # squared norms with XLA-matching rounding order: (x0^2 + x1^2) + x2^2
    sq = ((x[:, 0] * x[:, 0] + x[:, 1] * x[:, 1]) + x[:, 2] * x[:, 2]).astype(
        np.float32
    )
    r0 = half * rows
    rsl = slice(r0, r0 + rows)

    # K=30 bf16 operands; PSUM accumulates k sequentially so the order here
    # defines the fp32 rounding sequence (matches the verified emulation):
    # k = 9d+3i+j -> h_i(row coord d) * h_j(col coord d); k=27..29 -> 1 * m_j
    w30 = np.zeros((KSPLIT, rows), np.float32)
    b30 = np.zeros((KSPLIT, n), np.float32)
    for d in range(3):
        hr = _split3(x[rsl, d])
        hc = _split3(x[:, d])
        for i in range(3):
            for j in range(3):
                k = 9 * d + 3 * i + j
                w30[k] = hr[i]
                b30[k] = hc[j]
    m05 = (-0.5 * sq).astype(np.float32)
    ms = _split3(m05)
    for j in range(3):
        w30[27 + j] = 1.0
        b30[27 + j] = ms[j]
    w30 = w30.astype(ml_dtypes.bfloat16)
    b30 = b30.astype(ml_dtypes.bfloat16)

    thr = (0.5 * (sq[rsl] - np.float32(RADIUS2))).astype(np.float32)
    negthr = np.ascontiguousarray((-thr).reshape(nt, PT).T)

    # u16-plane packed gather table, SBUF stripe layout:
    # row idx -> partition idx%128, stripe idx//128; stripe = 256 u16 =
    # [low halves of 128 channels | high halves of 128 channels]
    rowdata = np.zeros((n, ELEM), np.float32)
    rowdata[:, :CFEAT] = feat_b
    rowdata[:, CFEAT : CFEAT + 3] = x
    u = rowdata.view(np.uint16).reshape(n, ELEM, 2)  # little-endian lo/hi
    planes = np.concatenate([u[:, :, 0], u[:, :, 1]], axis=1)  # [n, 256]
    table = np.zeros((PT, nrank * 2 * ELEM), np.uint16)
    for r in range(nrank):
        table[:, r * 2 * ELEM : (r + 1) * 2 * ELEM] = planes[
            r * PT : (r + 1) * PT
        ]

    vals = np.broadcast_to(
        np.asarray(SEARCH_VALS, np.float32)[None, :], (PT, 16)
    ).copy()
    rep = np.zeros((16, PT), np.float32)
    rep[np.arange(PT) % 16, np.arange(PT)] = 1.0
    ident = np.eye(PT, dtype=np.float32)

    return {
        "w30": w30,
        "b30": b30,
        "negthr": negthr,
        "table": table,
        "vals": vals,
        "rep": rep,
        "ident": ident,
    }


_NC_CACHE = {}


def _get_nc():
    if "nc" not in _NC_CACHE:
        _NC_CACHE["nc"] = build_nc()
    return _NC_CACHE["nc"]


def kernel(xyz, feature):
    """Full inputs -> full outputs (dilated_xyz [B,3,N,K], dilated_feature
    [B,CFEAT,N,K]), computed on 8 NeuronCores."""
    global LAST_RESULTS
    xyz = np.asarray(xyz)
    feature = np.asarray(feature)
    assert xyz.shape == (B, N, 3) and feature.shape == (B, N, CFEAT)

    nc = _get_nc()
    in_maps = [
        _host_prep(xyz[core // 2], feature[core // 2], core % 2)
        for core in range(N_CORES)
    ]
    trace = bool(int(os.environ.get("KNN_TRACE", "0")))
    if trace:
        # the axon trace path uploads artifacts to a fish bucket; neuter
        # that (no credentials in this container) so profiling is local-only
        import concourse.bass_utils as _bu

        _bu.upload_artifacts = lambda tmpdir: str(tmpdir)
    try:
        res = run_bass_kernel_spmd(
            nc, in_maps, core_ids=list(range(N_CORES)), trace=trace
        )
    except Exception:
        if not trace:
            raise
        res = run_bass_kernel_spmd(
            nc, in_maps, core_ids=list(range(N_CORES)), trace=False
        )
    LAST_RESULTS = res

    dxyz = np.empty((B, 3, N, K), np.float32)
    dfeat = np.empty((B, CFEAT, N, K), np.float32)
    for core in range(N_CORES):
        b, half = core // 2, core % 2
        o = res.results[core]["out"]  # [67, ROWS*K]
        rs = slice(half * ROWS, (half + 1) * ROWS)
        dfeat[b, :, rs, :] = o[:CFEAT].reshape(CFEAT, ROWS, K)
        dxyz[b, :, rs, :] = o[CFEAT : CFEAT + 3].reshape(3, ROWS, K)
    return dxyz, dfeat
